# revision 1
# baseline (speedup 1.0000x reference)
"""Masked Hillis-Steele scan kernel for Trainium2 (8 NeuronCores, SPMD).

Problem: B=131072 rows, L=512. For each row:
    y = where(mask, x, 0)
    for s in [1,2,4,...,512]:  # s=512 step is a no-op (shift of full row)
        y[i] += y[i-s]  if mask[i] and mask[i-s]

Key algebraic fact used: unmasked positions of y stay 0 forever, so
    mask[i-s]*y[i-s] == y[i-s]  and each step is  y += mask * shift_s(y).

Sharding: pure data parallel over B across the 8 cores.
"""

import os
import sys

import numpy as np

sys.path.insert(0, "/opt/trn_rl_repo")

B = 131072
L = 512
N_CORES = 8
BP = B // N_CORES  # rows per core = 16384

G = 1  # row-groups per macro tile
ROWS_PER_TILE = 128 * G  # 512
N_TILES = BP // ROWS_PER_TILE  # 32

_last_results = None  # stash for test harness introspection


def _legalize_waits(nc, cap=1):
    """Walrus's TRN2 instruction encodings only have room for a small number
    of sync-wait commands (1 for DMA/3D-AP tensor ops); Tile freely attaches
    more. Hoist surplus waits into standalone event-semaphore (wait-only)
    instructions inserted just before the over-subscribed instruction on the
    same engine queue."""
    import concourse.mybir as mybir

    n_new = 0
    for f in nc.m.functions:
        for b in f.blocks:
            new_list = []
            for ins in b.instructions:
                si = ins.sync_info
                if si is not None and len(si.on_wait) > cap:
                    waits = list(si.on_wait)
                    extra, keep = waits[:-cap], waits[-cap:]
                    for w in extra:
                        ev = mybir.InstEventSemaphore(
                            name=f"waitsplit_{n_new}", ins=[], outs=[]
                        )
                        ev.engine = ins.engine
                        ev.sync_info = mybir.SyncInfo(on_wait=[w], on_update=[])
                        new_list.append(ev)
                        n_new += 1
                    ins.sync_info = mybir.SyncInfo(
                        on_wait=keep, on_update=list(si.on_update)
                    )
                new_list.append(ins)
            b.instructions[:] = new_list
    return n_new


def _build_flat_program(reps=1, legalize=True):
    """Flat layout [rows on partitions, L on free dim]; all compute on DVE.

    reps>1 repeats the whole body (same I/O) for slope-based HW timing."""
    from contextlib import ExitStack

    import concourse.bass as bass
    import concourse.mybir as mybir
    import concourse.tile as tile

    f32 = mybir.dt.float32

    nc = bass.Bass(target_bir_lowering=False, debug=False)
    x_ext = nc.declare_dram_parameter("x", [BP, L], f32, isOutput=False)
    m_ext = nc.declare_dram_parameter("m", [BP, L], f32, isOutput=False)
    y_ext = nc.declare_dram_parameter("y", [BP, L], f32, isOutput=True)

    shifts = [1, 2, 4, 8, 16, 32, 64, 128, 256]

    with tile.TileContext(nc) as tc, ExitStack() as ctx:
        xp = ctx.enter_context(tc.tile_pool(name="xp", bufs=2))
        mp = ctx.enter_context(tc.tile_pool(name="mp", bufs=2))
        yp = ctx.enter_context(tc.tile_pool(name="yp", bufs=2))
        tp = ctx.enter_context(tc.tile_pool(name="tp", bufs=2))

        def body(_iv=None):
            for r in range(N_TILES):
                rows = slice(r * ROWS_PER_TILE, (r + 1) * ROWS_PER_TILE)
                xt = xp.tile([128, L], f32)
                mt = mp.tile([128, L], f32)
                yt = yp.tile([128, L], f32)
                tt = tp.tile([128, L], f32)

                nc.sync.dma_start(xt[:], x_ext[rows, :])
                nc.sync.dma_start(mt[:], m_ext[rows, :])

                # y = mask * x  (split as copy + in-place mul so each
                # instruction needs at most one DMA-queue wait: walrus's
                # 3D-AP TensorTensor encoding only fits a single sync wait)
                nc.vector.tensor_copy(yt[:], xt[:])
                nc.vector.tensor_mul(yt[:], yt[:], mt[:])
                for s in shifts:
                    # t[i] = m[i] * y[i-s]   for i in [s, L)
                    nc.vector.tensor_mul(tt[:, s:], yt[:, : L - s], mt[:, s:])
                    # y[i] += t[i]
                    nc.vector.tensor_add(yt[:, s:], yt[:, s:], tt[:, s:])

                nc.sync.dma_start(y_ext[rows, :], yt[:])

        if reps == 1:
            body()
        else:
            with tc.For_i(0, reps, 1) as iv:
                body(iv)

    if legalize:
        _legalize_waits(nc)
    return nc


def _shift_mats32():
    """32x32 stationary blocks for tile-packed PE shifts, replicated across
    the 4 partition row-strips: out[128, NMAT, 32], row p holds eye32[p%32].

    Block (source strip g -> output strip c) of the global shift-by-s
    operator is eye(32, k=d) with d = s - 32*(c - g)."""
    ds = [0, 1, 2, 4, 8, 16, -31, -30, -28, -24, -16]
    eyes = np.stack([np.eye(32, k=d, dtype=np.float32) for d in ds])  # [n,32,32]
    rep = np.concatenate([eyes] * 4, axis=1)  # [n,128,32]
    return np.ascontiguousarray(rep.transpose(1, 0, 2)), {d: i for i, d in enumerate(ds)}


N_SUPER = BP // 512  # 32 supertiles of 512 B-columns each
NMAT32 = 11


def _step_tiles(s):
    """[(out_strip c, src_strip g, eye-offset d)] for global shift-by-s."""
    out = []
    for c in range(16):
        lo = 32 * c - s
        gs = sorted({lo // 32, (lo + 31) // 32})
        for g in gs:
            if g < 0:
                continue
            d = s - 32 * (c - g)
            if -31 <= d <= 31:
                out.append((c, g, d))
    return out


_DBG_SHIFTS = None
_DBG_INTRA_ONLY = False


def _build_pe_program(reps=1, legalize=True):
    """Transposed layout: partitions = L (4 banks of 128), free = B columns.

    w (the unmasked running state) lives in PSUM; the TensorEngine applies
    w += S_s @ p_k (shift along L) via 32x32 tile-position-packed plain-fp32
    matmuls (bitwise exact for 0/1 matrices). The DVE computes only
    p_k = mask * w_k (PSUM -> SBUF), 10 passes per supertile."""
    from contextlib import ExitStack

    import concourse.bass as bass
    import concourse.mybir as mybir
    import concourse.tile as tile

    f32 = mybir.dt.float32

    nc = bass.Bass(target_bir_lowering=False, debug=False)
    x_ext = nc.declare_dram_parameter("xT", [L, BP], f32, isOutput=False)
    m_ext = nc.declare_dram_parameter("mT", [L, BP], f32, isOutput=False)
    mats_ext = nc.declare_dram_parameter(
        "mats32", [128, NMAT32, 32], f32, isOutput=False
    )
    y_ext = nc.declare_dram_parameter("yT", [L, BP], f32, isOutput=True)

    shifts = _DBG_SHIFTS if _DBG_SHIFTS is not None else [1, 2, 4, 8, 16, 32, 64, 128, 256]
    _, D2I = _shift_mats32()

    # last contributing step per output strip (for stop=True)
    last_s = {
        c: max(
            (s for s in shifts if any(t[0] == c for t in _step_tiles(s))),
            default=None,
        )
        for c in range(16)
    }

    NB = 512

    with tile.TileContext(nc) as tc, ExitStack() as ctx:
        cp = ctx.enter_context(tc.tile_pool(name="cp", bufs=1))
        xp = ctx.enter_context(tc.tile_pool(name="xp", bufs=2))
        mp = ctx.enter_context(tc.tile_pool(name="mp", bufs=2))
        pp = ctx.enter_context(tc.tile_pool(name="pp", bufs=4))
        op = ctx.enter_context(tc.tile_pool(name="op", bufs=2))
        dp = ctx.enter_context(tc.tile_pool(name="dp", bufs=2))
        wp = ctx.enter_context(tc.tile_pool(name="wp", bufs=2, space="PSUM"))

        mats_t = cp.tile([128, NMAT32, 32], f32)
        nc.sync.dma_start(mats_t[:], mats_ext[:])

        def lhs(row, d):
            # 32x32 stationary at partitions [row, row+32)
            return mats_t[row : row + 32, D2I[d], :]

        def body(_iv=None):
            for j in range(N_SUPER):
                cols = slice(j * NB, (j + 1) * NB)
                xt = xp.tile([128, 4, NB], f32)
                mt = mp.tile([128, 4, NB], f32)
                nc.sync.dma_start(
                    xt[:], x_ext[:, cols].rearrange("(blk p) b -> p blk b", p=128)
                )
                nc.sync.dma_start(
                    mt[:], m_ext[:, cols].rearrange("(blk p) b -> p blk b", p=128)
                )

                wt = wp.tile([128, 4, NB], f32)

                # p0 = mask * x  (SBUF x SBUF)
                pt = pp.tile([128, 4, NB], f32)
                nc.vector.tensor_mul(pt[:], xt[:], mt[:])

                def strip_out(c):
                    bank, row = divmod(c, 4)
                    return wt[:, bank, :][row * 32 : row * 32 + 32, :], row * 32

                def strip_in(t, g):
                    bank, row = divmod(g, 4)
                    return t[:, bank, :][row * 32 : row * 32 + 32, :], row * 32

                # init: w = x  (identity blocks, start=True)
                for c in range(16):
                    o, ocol = strip_out(c)
                    i, irow = strip_in(xt, c)
                    nc.tensor.matmul(
                        o, lhs(irow, 0), i,
                        start=True, stop=False, tile_position=(irow, ocol),
                        skip_group_check=True,
                    )

                for k, s in enumerate(shifts):
                    tiles = _step_tiles(s)
                    if _DBG_INTRA_ONLY:
                        tiles = [t for t in tiles if t[1] == t[0]]
                    # fp32 matmuls are two internal passes (LO/HI); two
                    # concurrently-executing tile-MMs accumulating into the
                    # same col strip corrupt/crash the PE. Emit the
                    # intra-strip wave, a PE drain, then the inter-strip
                    # wave so same-col pairs never overlap.
                    intra = [t for t in tiles if t[0] == t[1]]
                    inter = [t for t in tiles if t[0] != t[1]]
                    for wave_i, wave in enumerate((intra, inter)):
                        if wave_i == 1 and intra and inter:
                            # Serialize the two waves: an ACT read of the
                            # banks creates RAW (after intra) + WAR (before
                            # inter) edges, so same-col-strip fp32 tile-MMs
                            # never execute concurrently (HW corruption).
                            dummy = dp.tile([128, 4, 1], f32)
                            nc.scalar.copy(dummy[:], wt[:, :, 0:1])
                        for idx, (c, g, d) in enumerate(wave):
                            o, ocol = strip_out(c)
                            i, irow = strip_in(pt, g)
                            is_last = s == last_s[c] and not any(
                                t[0] == c for t in (wave[idx + 1 :] + (inter if wave_i == 0 else []))
                            )
                            nc.tensor.matmul(
                                o, lhs(irow, d), i,
                                start=False, stop=is_last, tile_position=(irow, ocol),
                                skip_group_check=True,
                            )
                    # DVE: p_{k+1} = mask * w  (PSUM x SBUF -> SBUF)
                    if s != shifts[-1]:
                        pt = pp.tile([128, 4, NB], f32)
                        nc.vector.tensor_mul(pt[:], wt[:], mt[:])
                    else:
                        ot = op.tile([128, 4, NB], f32)
                        nc.vector.tensor_mul(ot[:], wt[:], mt[:])
                        nc.sync.dma_start(
                            y_ext[:, cols].rearrange(
                                "(blk p) b -> p blk b", p=128
                            ),
                            ot[:],
                        )

        if reps == 1:
            body()
        else:
            with tc.For_i(0, reps, 1) as iv:
                body(iv)

    if legalize:
        _legalize_waits(nc)
    return nc


_cached = {}


def kernel(x, mask):
    global _last_results
    from concourse.bass_utils import run_bass_kernel_spmd

    mode = os.environ.get("KERNEL_MODE", "pe")
    x = np.ascontiguousarray(np.asarray(x, dtype=np.float32))
    m = np.asarray(mask)
    assert x.shape == (B, L) and m.shape == (B, L)
    m_f = m.astype(np.float32)

    if mode not in _cached:
        _cached[mode] = (
            _build_pe_program() if mode == "pe" else _build_flat_program()
        )
    nc = _cached[mode]

    core_ids = list(range(N_CORES))
    if mode == "pe":
        mats32, _ = _shift_mats32()
        in_maps = [
            {
                "xT": np.ascontiguousarray(x[i * BP : (i + 1) * BP].T),
                "mT": np.ascontiguousarray(m_f[i * BP : (i + 1) * BP].T),
                "mats32": mats32,
            }
            for i in core_ids
        ]
    else:
        in_maps = [
            {
                "x": x[i * BP : (i + 1) * BP],
                "m": np.ascontiguousarray(m_f[i * BP : (i + 1) * BP]),
            }
            for i in core_ids
        ]

    res = run_bass_kernel_spmd(nc, in_maps, core_ids)
    _last_results = res

    out = np.empty((B, L), dtype=np.float32)
    for i in core_ids:
        if mode == "pe":
            out[i * BP : (i + 1) * BP] = res.results[i]["yT"].T
        else:
            out[i * BP : (i + 1) * BP] = res.results[i]["y"]
    return out



# revision 2
# speedup vs baseline: 2.0794x; 2.0794x over previous
"""Masked Hillis-Steele scan kernel for Trainium2 (8 NeuronCores, SPMD).

Problem: B=131072 rows, L=512. For each row:
    y = where(mask, x, 0)
    for s in [1,2,4,...,512]:  # s=512 step is a no-op (shift of full row)
        y[i] += y[i-s]  if mask[i] and mask[i-s]

Key algebraic fact: unmasked positions of y stay 0 forever, so
    mask[i-s]*y[i-s] == y[i-s]  and each step is  y += mask * shift_s(y).

Design (flat/DVE): rows on partitions, L on the free dim, fp16 on-chip.
The shift is a free-dim AP offset (no PE/PSUM involved). Every
elementwise op is emitted as scalar_tensor_tensor (InstTensorScalarPtr),
which supports the DVE 4x perf mode (2x for packed 16-bit operands x 2x
for all-SBUF operands) -> ~0.26 ns per free element. Mask is DMA'd as
uint8 (1/4 the bytes) and cast to fp16 on the otherwise-idle Act engine.

Sharding: pure data parallel over B across the 8 cores.
"""

import os
import sys

import numpy as np

sys.path.insert(0, "/opt/trn_rl_repo")

B = 131072
L = 512
N_CORES = 8
BP = B // N_CORES  # rows per core = 16384

G = 16  # row-groups per tile: tile = [128 partitions, G groups, L]
ROWS_PER_TILE = 128 * G  # 2048
N_TILES = BP // ROWS_PER_TILE  # 8

SHIFTS = [1, 2, 4, 8, 16, 32, 64, 128, 256]

_last_results = None  # stash for test harness introspection


def _legalize_waits(nc, cap=1):
    """Walrus's TRN2 instruction encodings only have room for a small number
    of sync-wait commands (1 for DMA/3D-AP tensor ops); Tile freely attaches
    more. Hoist surplus waits into standalone event-semaphore (wait-only)
    instructions inserted just before the over-subscribed instruction on the
    same engine queue."""
    import concourse.mybir as mybir

    n_new = 0
    for f in nc.m.functions:
        for b in f.blocks:
            new_list = []
            for ins in b.instructions:
                si = ins.sync_info
                if si is not None and len(si.on_wait) > cap:
                    waits = list(si.on_wait)
                    extra, keep = waits[:-cap], waits[-cap:]
                    for w in extra:
                        ev = mybir.InstEventSemaphore(
                            name=f"waitsplit_{n_new}", ins=[], outs=[]
                        )
                        ev.engine = ins.engine
                        ev.sync_info = mybir.SyncInfo(on_wait=[w], on_update=[])
                        new_list.append(ev)
                        n_new += 1
                    ins.sync_info = mybir.SyncInfo(
                        on_wait=keep, on_update=list(si.on_update)
                    )
                new_list.append(ins)
            b.instructions[:] = new_list
    return n_new


def _build_flat16_program(reps=1, legalize=True):
    """Flat layout [128, G, L] fp16; all compute on DVE via
    scalar_tensor_tensor (4x perf mode); mask u8->fp16 cast on Act.

    reps>1 repeats the whole body (same I/O) for slope-based HW timing."""
    from contextlib import ExitStack

    import concourse.bass as bass
    import concourse.mybir as mybir
    import concourse.tile as tile

    f16 = mybir.dt.float16
    u8 = mybir.dt.uint8
    MUL = mybir.AluOpType.mult
    ADD = mybir.AluOpType.add

    nc = bass.Bass(target_bir_lowering=False, debug=False)
    x_ext = nc.declare_dram_parameter("x", [BP, L], f16, isOutput=False)
    m_ext = nc.declare_dram_parameter("m", [BP, L], u8, isOutput=False)
    y_ext = nc.declare_dram_parameter("y", [BP, L], f16, isOutput=True)

    with tile.TileContext(nc) as tc, ExitStack() as ctx:
        xp = ctx.enter_context(tc.tile_pool(name="xp", bufs=2))
        m8p = ctx.enter_context(tc.tile_pool(name="m8p", bufs=2))
        mp = ctx.enter_context(tc.tile_pool(name="mp", bufs=2))
        yp = ctx.enter_context(tc.tile_pool(name="yp", bufs=2))
        tp = ctx.enter_context(tc.tile_pool(name="tp", bufs=2))

        def body(_iv=None):
            for r in range(N_TILES):
                rows = slice(r * ROWS_PER_TILE, (r + 1) * ROWS_PER_TILE)
                xt = xp.tile([128, G, L], f16)
                m8 = m8p.tile([128, G, L], u8)
                mt = mp.tile([128, G, L], f16)
                yt = yp.tile([128, G, L], f16)
                tt = tp.tile([128, G, L], f16)

                # DRAM row g*128+p -> tile[p, g, :]
                nc.sync.dma_start(
                    xt[:], x_ext[rows, :].rearrange("(g p) l -> p g l", p=128)
                )
                nc.sync.dma_start(
                    m8[:], m_ext[rows, :].rearrange("(g p) l -> p g l", p=128)
                )

                # Act engine: cast mask u8 -> fp16 (keeps DVE free)
                nc.scalar.copy(mt[:], m8[:])

                # y = (x * 1.0) * m   -- stt runs in DVE 4x mode
                nc.vector.scalar_tensor_tensor(yt[:], xt[:], 1.0, mt[:], MUL, MUL)

                for s in SHIFTS:
                    # t[i] = y[i-s] * m[i]   for i in [s, L)
                    nc.vector.scalar_tensor_tensor(
                        tt[:, :, s:], yt[:, :, : L - s], 1.0, mt[:, :, s:], MUL, MUL
                    )
                    # y[i] = (y[i] * 1.0) + t[i]
                    nc.vector.scalar_tensor_tensor(
                        yt[:, :, s:], yt[:, :, s:], 1.0, tt[:, :, s:], MUL, ADD
                    )

                nc.sync.dma_start(
                    y_ext[rows, :].rearrange("(g p) l -> p g l", p=128), yt[:]
                )

        if reps == 1:
            body()
        else:
            with tc.For_i(0, reps, 1) as iv:
                body(iv)

    if legalize:
        _legalize_waits(nc)
    return nc


_cached = {}


def kernel(x, mask):
    global _last_results
    from concourse.bass_utils import run_bass_kernel_spmd

    x = np.asarray(x)
    m = np.asarray(mask)
    assert x.shape == (B, L) and m.shape == (B, L)
    x16 = x.astype(np.float16)
    m8 = m.astype(np.uint8)

    if "flat16" not in _cached:
        _cached["flat16"] = _build_flat16_program()
    nc = _cached["flat16"]

    core_ids = list(range(N_CORES))
    in_maps = [
        {
            "x": x16[i * BP : (i + 1) * BP],
            "m": m8[i * BP : (i + 1) * BP],
        }
        for i in core_ids
    ]

    res = run_bass_kernel_spmd(nc, in_maps, core_ids)
    _last_results = res

    out = np.empty((B, L), dtype=np.float32)
    for i in core_ids:
        out[i * BP : (i + 1) * BP] = res.results[i]["y"].astype(np.float32)
    return out


# revision 3
# speedup vs baseline: 3.9459x; 1.8976x over previous
"""Masked Hillis-Steele scan kernel for Trainium2 (8 NeuronCores, SPMD).

Problem: B=131072 rows, L=512. For each row:
    y = where(mask, x, 0)
    for s in [1,2,4,...,512]:  # s=512 step is a no-op (shift of full row)
        y[i] += y[i-s]  if mask[i] and mask[i-s]

Key algebraic fact: unmasked positions of y stay 0 forever, so
    mask[i-s]*y[i-s] == y[i-s]  and each step is  y += mask * shift_s(y).

Design (flat/DVE): rows on partitions, L on the free dim, fp16 on-chip.
The shift is a free-dim AP offset (no PE/PSUM involved). Every
elementwise op is emitted as scalar_tensor_tensor (InstTensorScalarPtr),
which supports the DVE 4x perf mode (2x for packed 16-bit operands x 2x
for all-SBUF operands) -> ~0.26 ns per free element. Mask is DMA'd as
uint8 (1/4 the bytes) and cast to fp16 on the otherwise-idle Act engine.

Sharding: pure data parallel over B across the 8 cores.
"""

import os
import sys

import numpy as np

sys.path.insert(0, "/opt/trn_rl_repo")

B = 131072
L = 512
N_CORES = 8
BP = B // N_CORES  # rows per core = 16384

G = 16  # row-groups per tile: tile = [128 partitions, G groups, L]
ROWS_PER_TILE = 128 * G  # 2048
N_TILES = BP // ROWS_PER_TILE  # 8

SHIFTS = [1, 2, 4, 8, 16, 32, 64, 128, 256]

_last_results = None  # stash for test harness introspection


def _legalize_waits(nc, cap=1):
    """Walrus's TRN2 instruction encodings only have room for a small number
    of sync-wait commands (1 for DMA/3D-AP tensor ops); Tile freely attaches
    more. Hoist surplus waits into standalone event-semaphore (wait-only)
    instructions inserted just before the over-subscribed instruction on the
    same engine queue."""
    import concourse.mybir as mybir

    n_new = 0
    for f in nc.m.functions:
        for b in f.blocks:
            new_list = []
            for ins in b.instructions:
                si = ins.sync_info
                if si is not None and len(si.on_wait) > cap:
                    waits = list(si.on_wait)
                    extra, keep = waits[:-cap], waits[-cap:]
                    for w in extra:
                        ev = mybir.InstEventSemaphore(
                            name=f"waitsplit_{n_new}", ins=[], outs=[]
                        )
                        ev.engine = ins.engine
                        ev.sync_info = mybir.SyncInfo(on_wait=[w], on_update=[])
                        new_list.append(ev)
                        n_new += 1
                    ins.sync_info = mybir.SyncInfo(
                        on_wait=keep, on_update=list(si.on_update)
                    )
                new_list.append(ins)
            b.instructions[:] = new_list
    return n_new


def _build_flat16_program(reps=1, legalize=True):
    """Flat layout [128, G, L] fp16; all compute on DVE via
    scalar_tensor_tensor (4x perf mode); mask u8->fp16 cast on Act.

    reps>1 repeats the whole body (same I/O) for slope-based HW timing."""
    from contextlib import ExitStack

    import concourse.bass as bass
    import concourse.mybir as mybir
    import concourse.tile as tile

    f16 = mybir.dt.float16
    u8 = mybir.dt.uint8
    MUL = mybir.AluOpType.mult
    ADD = mybir.AluOpType.add

    nc = bass.Bass(target_bir_lowering=False, debug=False)
    x_ext = nc.declare_dram_parameter("x", [BP, L], f16, isOutput=False)
    m_ext = nc.declare_dram_parameter("m", [BP, L], u8, isOutput=False)
    y_ext = nc.declare_dram_parameter("y", [BP, L], f16, isOutput=True)

    with tile.TileContext(nc) as tc, ExitStack() as ctx:
        xp = ctx.enter_context(tc.tile_pool(name="xp", bufs=2))
        m8p = ctx.enter_context(tc.tile_pool(name="m8p", bufs=2))
        mp = ctx.enter_context(tc.tile_pool(name="mp", bufs=2))
        yp = ctx.enter_context(tc.tile_pool(name="yp", bufs=2))
        tp = ctx.enter_context(tc.tile_pool(name="tp", bufs=2))

        def body(_iv=None):
            for r in range(N_TILES):
                rows = slice(r * ROWS_PER_TILE, (r + 1) * ROWS_PER_TILE)
                xt = xp.tile([128, G, L], f16)
                m8 = m8p.tile([128, G, L], u8)
                mt = mp.tile([128, G, L], f16)
                yt = yp.tile([128, G, L], f16)
                tt = tp.tile([128, G, L], f16)

                # DRAM row g*128+p -> tile[p, g, :]
                nc.sync.dma_start(
                    xt[:], x_ext[rows, :].rearrange("(g p) l -> p g l", p=128)
                )
                nc.sync.dma_start(
                    m8[:], m_ext[rows, :].rearrange("(g p) l -> p g l", p=128)
                )

                # Act engine: cast mask u8 -> fp16 (keeps DVE free)
                nc.scalar.copy(mt[:], m8[:])

                # y = x * m   (TensorTensor: 2x_1p perf mode for packed fp16)
                nc.vector.tensor_mul(yt[:], xt[:], mt[:])

                for s in SHIFTS:
                    # t[i] = y[i-s] * m[i]   for i in [s, L)
                    nc.vector.tensor_mul(
                        tt[:, :, s:], yt[:, :, : L - s], mt[:, :, s:]
                    )
                    # y[i] += t[i]
                    nc.vector.tensor_add(
                        yt[:, :, s:], yt[:, :, s:], tt[:, :, s:]
                    )

                nc.sync.dma_start(
                    y_ext[rows, :].rearrange("(g p) l -> p g l", p=128), yt[:]
                )

        if reps == 1:
            body()
        else:
            with tc.For_i(0, reps, 1) as iv:
                body(iv)

    if legalize:
        _legalize_waits(nc)
    return nc


_cached = {}


def kernel(x, mask):
    global _last_results
    from concourse.bass_utils import run_bass_kernel_spmd

    x = np.asarray(x)
    m = np.asarray(mask)
    assert x.shape == (B, L) and m.shape == (B, L)
    x16 = x.astype(np.float16)
    m8 = m.astype(np.uint8)

    if "flat16" not in _cached:
        _cached["flat16"] = _build_flat16_program()
    nc = _cached["flat16"]

    core_ids = list(range(N_CORES))
    in_maps = [
        {
            "x": x16[i * BP : (i + 1) * BP],
            "m": m8[i * BP : (i + 1) * BP],
        }
        for i in core_ids
    ]

    res = run_bass_kernel_spmd(nc, in_maps, core_ids)
    _last_results = res

    out = np.empty((B, L), dtype=np.float32)
    for i in core_ids:
        out[i * BP : (i + 1) * BP] = res.results[i]["y"].astype(np.float32)
    return out
